# revision 15
# baseline (speedup 1.0000x reference)
"""LoRA linear (y = x @ (W + s*B@A)^T + bias) on 8 Trainium2 NeuronCores.

Strategy: pure data parallel over the token dim. The LoRA update is folded
into the weight on the host (W' = W + 4.0 * B @ A, rank-8 update, ~17 MFLOP
in numpy), so the device kernel is a plain fp32 linear. x is transposed on
the host so the contraction dim (d) lands on SBUF partitions with fully
contiguous DMA lines. Matmuls run as float32r (full fp32 storage, fast PE
mode, 1 col/cycle for moving dim >= 256).

Per core: out[2048, 1024] = xT[:, shard].T @ wT + bias
  - wT [1024(d), 1024(o)] resident in SBUF (4 MiB), loaded once
  - x streamed in 512-token blocks [128(d-part), 8(d-tile), 512(n)]
  - psum [128(n), 1024(o)] accumulated over 8 d-tiles, 2 o-halves of 512
  - DVE adds bias (broadcast into [128, 1024] SBUF once) on PSUM eviction
"""

import os
import sys

import numpy as np

for _p in ("/opt/trn_rl_repo", "/opt/pypackages"):
    if os.path.isdir(_p) and _p not in sys.path:
        sys.path.append(_p)

try:
    import jax

    jax.config.update(
        "jax_compilation_cache_dir", os.path.expanduser("~/.cache/jax_bass_cache")
    )
    jax.config.update("jax_persistent_cache_min_compile_time_secs", 0.0)
except Exception:
    pass

import concourse.bass as bass  # noqa: E402,F401
import concourse.mybir as mybir  # noqa: E402
import concourse.tile as tile  # noqa: E402
from concourse import bacc  # noqa: E402
from concourse.bass_utils import run_bass_kernel_spmd  # noqa: E402

N_CORES = 8
N_TOK, D_IN, D_OUT = 16384, 1024, 1024
N_SHARD = N_TOK // N_CORES  # 2048 tokens per core
P = 128
SCALING = 4.0  # alpha / r = 32 / 8

_CACHE: dict = {}


def build_nc():
    f32 = mybir.dt.float32
    f32r = mybir.dt.float32r
    nc = bacc.Bacc("TRN2", target_bir_lowering=False, debug=False)

    # float32r: same 4-byte storage as fp32 (numpy float32), but runs the PE
    # at 1 col/cycle instead of fp32's 4. The BIR verifier requires matmult
    # inputs to be produced as fp32r, so declare them fp32r end-to-end.
    xT = nc.dram_tensor("xT", [D_IN, N_SHARD], f32r, kind="ExternalInput")
    wT = nc.dram_tensor("wT", [D_IN, D_OUT], f32r, kind="ExternalInput")
    bias = nc.dram_tensor("bias", [1, D_OUT], f32, kind="ExternalInput")
    out = nc.dram_tensor("out", [N_SHARD, D_OUT], f32, kind="ExternalOutput")

    KT = D_IN // P  # 8 contraction tiles
    NBLK = 512  # tokens per group (4 psum tiles of 128)
    GRP = NBLK // P  # 4 psum tiles accumulated concurrently (8 banks)
    OH = 512  # max fp32 moving free dim (one PSUM bank)

    NGRP = N_SHARD // NBLK
    with tile.TileContext(nc) as tc:
        with tc.tile_pool(name="const", bufs=1) as const_pool, \
                tc.tile_pool(name="xp", bufs=2 * KT) as x_pool, \
                tc.tile_pool(name="op", bufs=4) as out_pool, \
                tc.tile_pool(name="ps", bufs=GRP, space="PSUM") as psum_pool:
            # Weight resident in SBUF, one tile per d-tile; x streamed as one
            # tile per (group, d-tile) so every matmul waits only on its own
            # 256-512KB DMA. DMA issue (~0.65us serial per dma_start per
            # sequencer) is spread across four engines: w on gpsimd+scalar,
            # x on sync+vector, out on gpsimd.
            w_tiles = []
            for t in range(KT):
                eng = nc.gpsimd if t < KT // 2 else nc.scalar
                w_t = const_pool.tile([P, D_OUT], f32r, name=f"w{t}")
                eng.dma_start(w_t[:], wT[t * P:(t + 1) * P, :])
                w_tiles.append(w_t)
            bias_sb = const_pool.tile([P, D_OUT], f32)
            nc.scalar.dma_start(bias_sb[:], bias[:].to_broadcast((P, D_OUT)))

            def load_x(g):
                tiles = []
                for t in range(KT):
                    # split issue across two sequencers (~0.7us serial each)
                    eng = nc.sync if t % 2 == 0 else nc.scalar
                    x_t = x_pool.tile([P, NBLK], f32r, name=f"x_g{g}_d{t}",
                                      tag="xd")
                    eng.dma_start(
                        x_t[:], xT[t * P:(t + 1) * P, g * NBLK:(g + 1) * NBLK]
                    )
                    tiles.append(x_t)
                return tiles

            x_tiles = {0: load_x(0)}

            def evict(g, i, psum):
                o_sb = out_pool.tile([P, D_OUT], f32)
                nc.vector.tensor_add(o_sb[:], psum[:], bias_sb[:])
                n0 = g * NBLK + i * P
                nc.gpsimd.dma_start(out[n0:n0 + P, :], o_sb[:])

            for g in range(NGRP):
                if g + 1 < NGRP:
                    x_tiles[g + 1] = load_x(g + 1)
                xt = x_tiles.pop(g)
                psums = [
                    psum_pool.tile([P, D_OUT], f32, name=f"ps_g{g}_{i}",
                                   tag="psum")
                    for i in range(GRP)
                ]
                if g < NGRP - 1:
                    # d-outer: each arriving (w[d], x[d]) slice immediately
                    # enables 8 matmuls while later slices are in flight.
                    for d in range(KT):
                        for i in range(GRP):
                            lhsT = xt[d][:, i * P:(i + 1) * P]
                            for h in range(D_OUT // OH):
                                nc.tensor.matmul(
                                    psums[i][:, h * OH:(h + 1) * OH],
                                    lhsT,
                                    w_tiles[d][:, h * OH:(h + 1) * OH],
                                    start=(d == 0),
                                    stop=(d == KT - 1),
                                )
                    for i in range(GRP):
                        evict(g, i, psums[i])
                else:
                    # last group, data resident: i-outer spreads psum
                    # completions so the tail isn't 4 serialized evictions.
                    for i in range(GRP):
                        for d in range(KT):
                            lhsT = xt[d][:, i * P:(i + 1) * P]
                            for h in range(D_OUT // OH):
                                nc.tensor.matmul(
                                    psums[i][:, h * OH:(h + 1) * OH],
                                    lhsT,
                                    w_tiles[d][:, h * OH:(h + 1) * OH],
                                    start=(d == 0),
                                    stop=(d == KT - 1),
                                )
                        evict(g, i, psums[i])

    nc.finalize()
    return nc


def _get_nc():
    if "nc" not in _CACHE:
        _CACHE["nc"] = build_nc()
    return _CACHE["nc"]


def kernel(x, weight, bias, A, B):
    x = np.asarray(x, dtype=np.float32)
    weight = np.asarray(weight, dtype=np.float32)
    bias = np.asarray(bias, dtype=np.float32)
    A = np.asarray(A, dtype=np.float32)
    B = np.asarray(B, dtype=np.float32)

    # Fold the rank-8 LoRA update into the weight (exact up to fp32 rounding).
    w_eff = (
        weight.astype(np.float64) + SCALING * (B.astype(np.float64) @ A.astype(np.float64))
    ).astype(np.float32)
    wT = np.ascontiguousarray(w_eff.T)  # [d, o]
    xT = np.ascontiguousarray(x.T)  # [d, n]
    bias2d = np.ascontiguousarray(bias.reshape(1, D_OUT))

    nc = _get_nc()
    in_maps = [
        {
            "xT": np.ascontiguousarray(xT[:, c * N_SHARD:(c + 1) * N_SHARD]),
            "wT": wT,
            "bias": bias2d,
        }
        for c in range(N_CORES)
    ]
    trace_kwargs = {}
    if os.environ.get("KERNEL_TRACE") == "1":
        trace_kwargs = {"trace": True}
    res = run_bass_kernel_spmd(nc, in_maps, list(range(N_CORES)), **trace_kwargs)
    _CACHE["last_results"] = res
    return np.concatenate([r["out"] for r in res.results], axis=0)


# revision 16
# speedup vs baseline: 1.1115x; 1.1115x over previous
"""LoRA linear (y = x @ (W + s*B@A)^T + bias) on 8 Trainium2 NeuronCores.

Strategy: pure data parallel over the token dim. The LoRA update is folded
into the weight on the host (W' = W + 4.0 * B @ A, rank-8 update, ~17 MFLOP
in numpy), so the device kernel is a plain fp32 linear. x is transposed on
the host so the contraction dim (d) lands on SBUF partitions with fully
contiguous DMA lines. Matmuls run as float32r (full fp32 storage, fast PE
mode, 1 col/cycle for moving dim >= 256).

Per core: out[2048, 1024] = xT[:, shard].T @ wT + bias
  - wT [1024(d), 1024(o)] resident in SBUF (4 MiB), loaded once
  - x streamed in 512-token blocks [128(d-part), 8(d-tile), 512(n)]
  - psum [128(n), 1024(o)] accumulated over 8 d-tiles, 2 o-halves of 512
  - DVE adds bias (broadcast into [128, 1024] SBUF once) on PSUM eviction
"""

import os
import sys

import numpy as np

for _p in ("/opt/trn_rl_repo", "/opt/pypackages"):
    if os.path.isdir(_p) and _p not in sys.path:
        sys.path.append(_p)

try:
    import jax

    jax.config.update(
        "jax_compilation_cache_dir", os.path.expanduser("~/.cache/jax_bass_cache")
    )
    jax.config.update("jax_persistent_cache_min_compile_time_secs", 0.0)
except Exception:
    pass

import concourse.bass as bass  # noqa: E402,F401
import concourse.mybir as mybir  # noqa: E402
import concourse.tile as tile  # noqa: E402
from concourse import bacc  # noqa: E402
from concourse.bass_utils import run_bass_kernel_spmd  # noqa: E402

N_CORES = 8
N_TOK, D_IN, D_OUT = 16384, 1024, 1024
N_SHARD = N_TOK // N_CORES  # 2048 tokens per core
P = 128
SCALING = 4.0  # alpha / r = 32 / 8

_CACHE: dict = {}


def build_nc():
    f32 = mybir.dt.float32
    f32r = mybir.dt.float32r
    nc = bacc.Bacc("TRN2", target_bir_lowering=False, debug=False)

    # float32r: same 4-byte storage as fp32 (numpy float32), but runs the PE
    # at 1 col/cycle instead of fp32's 4. The BIR verifier requires matmult
    # inputs to be produced as fp32r, so declare them fp32r end-to-end.
    xT = nc.dram_tensor("xT", [D_IN, N_SHARD], f32r, kind="ExternalInput")
    wT = nc.dram_tensor("wT", [D_IN, D_OUT], f32r, kind="ExternalInput")
    bias = nc.dram_tensor("bias", [1, D_OUT], f32, kind="ExternalInput")
    out = nc.dram_tensor("out", [N_SHARD, D_OUT], f32, kind="ExternalOutput")

    KT = D_IN // P  # 8 contraction tiles
    NBLK = 512  # tokens per group (4 psum tiles of 128)
    GRP = NBLK // P  # 4 psum tiles accumulated concurrently (8 banks)
    OH = 512  # max fp32 moving free dim (one PSUM bank)

    NGRP = N_SHARD // NBLK
    XBLK = 2 * NBLK  # 1024 tokens per x tile: 4KB DMA lines, serves 2 groups
    with tile.TileContext(nc) as tc:
        with tc.tile_pool(name="const", bufs=1) as const_pool, \
                tc.tile_pool(name="xp", bufs=2 * KT) as x_pool, \
                tc.tile_pool(name="op", bufs=4) as out_pool, \
                tc.tile_pool(name="ps", bufs=GRP, space="PSUM") as psum_pool:
            # DMA queues drain lines in global issue order, so ALL input DMAs
            # go on one sequencer (sync) in exact consumption order:
            # (w0,x0), (w1,x1), ..., bias, then later x super-blocks. The PE
            # starts after the first ~1MiB pair instead of the whole 8.5MiB
            # fill. Out DMAs ride gpsimd, evictions ride vector.
            w_tiles = [
                const_pool.tile([P, D_OUT], f32r, name=f"w{t}")
                for t in range(KT)
            ]
            bias_sb = const_pool.tile([P, D_OUT], f32)

            def load_x_super(gg):
                tiles = []
                for t in range(KT):
                    x_t = x_pool.tile([P, XBLK], f32r, name=f"x_gg{gg}_d{t}",
                                      tag="xd")
                    nc.sync.dma_start(
                        x_t[:],
                        xT[t * P:(t + 1) * P, gg * XBLK:(gg + 1) * XBLK],
                    )
                    tiles.append(x_t)
                return tiles

            # startup stream in consumption order: w[d] paired with x[d]
            x_super = {}
            xs0 = []
            for t in range(KT):
                nc.sync.dma_start(w_tiles[t][:], wT[t * P:(t + 1) * P, :])
                x_t = x_pool.tile([P, XBLK], f32r, name=f"x_gg0_d{t}",
                                  tag="xd")
                nc.sync.dma_start(x_t[:], xT[t * P:(t + 1) * P, 0:XBLK])
                xs0.append(x_t)
            x_super[0] = xs0
            nc.sync.dma_start(bias_sb[:], bias[:].to_broadcast((P, D_OUT)))

            def evict(g, i, psum):
                o_sb = out_pool.tile([P, D_OUT], f32)
                nc.vector.tensor_add(o_sb[:], psum[:], bias_sb[:])
                n0 = g * NBLK + i * P
                nc.gpsimd.dma_start(out[n0:n0 + P, :], o_sb[:])

            for g in range(NGRP):
                gg, half = divmod(g, 2)
                if g % 2 == 0 and gg + 1 < NGRP // 2:
                    x_super[gg + 1] = load_x_super(gg + 1)
                xt = [
                    x_super[gg][t][:, half * NBLK:(half + 1) * NBLK]
                    for t in range(KT)
                ]
                psums = [
                    psum_pool.tile([P, D_OUT], f32, name=f"ps_g{g}_{i}",
                                   tag="psum")
                    for i in range(GRP)
                ]
                if g < NGRP - 1:
                    # d-outer: each arriving (w[d], x[d]) slice immediately
                    # enables 8 matmuls while later slices are in flight.
                    for d in range(KT):
                        for i in range(GRP):
                            lhsT = xt[d][:, i * P:(i + 1) * P]
                            for h in range(D_OUT // OH):
                                nc.tensor.matmul(
                                    psums[i][:, h * OH:(h + 1) * OH],
                                    lhsT,
                                    w_tiles[d][:, h * OH:(h + 1) * OH],
                                    start=(d == 0),
                                    stop=(d == KT - 1),
                                )
                    for i in range(GRP):
                        evict(g, i, psums[i])
                else:
                    # last group, data resident: i-outer spreads psum
                    # completions so the tail isn't 4 serialized evictions.
                    for i in range(GRP):
                        for d in range(KT):
                            lhsT = xt[d][:, i * P:(i + 1) * P]
                            for h in range(D_OUT // OH):
                                nc.tensor.matmul(
                                    psums[i][:, h * OH:(h + 1) * OH],
                                    lhsT,
                                    w_tiles[d][:, h * OH:(h + 1) * OH],
                                    start=(d == 0),
                                    stop=(d == KT - 1),
                                )
                        evict(g, i, psums[i])

    nc.finalize()
    return nc


def _get_nc():
    if "nc" not in _CACHE:
        _CACHE["nc"] = build_nc()
    return _CACHE["nc"]


def kernel(x, weight, bias, A, B):
    x = np.asarray(x, dtype=np.float32)
    weight = np.asarray(weight, dtype=np.float32)
    bias = np.asarray(bias, dtype=np.float32)
    A = np.asarray(A, dtype=np.float32)
    B = np.asarray(B, dtype=np.float32)

    # Fold the rank-8 LoRA update into the weight (exact up to fp32 rounding).
    w_eff = (
        weight.astype(np.float64) + SCALING * (B.astype(np.float64) @ A.astype(np.float64))
    ).astype(np.float32)
    wT = np.ascontiguousarray(w_eff.T)  # [d, o]
    xT = np.ascontiguousarray(x.T)  # [d, n]
    bias2d = np.ascontiguousarray(bias.reshape(1, D_OUT))

    nc = _get_nc()
    in_maps = [
        {
            "xT": np.ascontiguousarray(xT[:, c * N_SHARD:(c + 1) * N_SHARD]),
            "wT": wT,
            "bias": bias2d,
        }
        for c in range(N_CORES)
    ]
    trace_kwargs = {}
    if os.environ.get("KERNEL_TRACE") == "1":
        trace_kwargs = {"trace": True}
    res = run_bass_kernel_spmd(nc, in_maps, list(range(N_CORES)), **trace_kwargs)
    _CACHE["last_results"] = res
    return np.concatenate([r["out"] for r in res.results], axis=0)
